# revision 1
# baseline (speedup 1.0000x reference)
"""MixGCN (2-layer GCN with GReLU mix) on 8 Trainium2 NeuronCores.

Sharding: nodes partitioned by destination across 8 cores (graph parallel).
  L1: every core computes the full table g1 = dinv * (x @ W1) (replicated
      dense matmul -- cheaper than AllGathering the 51MB table), then
      aggregates its own 1/8 of destinations via dma_gather of source rows +
      one-hot-matmul segment sums in PSUM; the epilogue applies dinv[dst],
      bias, and the GReLU mix.
  L2: each core computes g2 = dinv * (h @ W2) for its own rows, AllGathers
      the 25.6MB table, aggregates its destinations the same way, and writes
      its output shard.
Self-loops are folded into the edge list on the host. All loop bounds are
static and identical across cores (SPMD): per-cell slot counts are the max
over cores; padded slots gather row 0 with an out-of-range dest id so their
one-hot rows are all-zero.
"""
import json
import math
import os

import numpy as np

P = 128
NCORES = 8
GROUP = 4          # dest tiles per gather group (PSUM: 4 live + 4 rotating)
OOB = 1.0e6        # dest-local sentinel for padded slots
BETA, CMIX = 0.5, 1.0

_EXEC_STATS = {}   # the test harness reads timing info from here


# --------------------------------------------------------------------------
# Workaround for this walrus build's per-instruction sync-wait limit: hoist
# excess immediate semaphore waits onto NoOps inserted before the offending
# instruction on the same engine (sem values are monotonic in-kernel, so
# waiting earlier is equivalent).
# --------------------------------------------------------------------------
_MAXW = 1


def _split_excess_waits(bir: dict) -> dict:
    ctr = 0
    for fn in bir.get("functions", []):
        for bb in fn.get("blocks", []):
            insts = bb.get("instructions", [])
            if not any(
                len(((i.get("sync_info") or {}).get("on_wait") or [])) > _MAXW
                for i in insts
            ):
                continue
            new_insts = []
            for inst in insts:
                si = inst.get("sync_info") or {}
                waits = si.get("on_wait") or []
                if len(waits) > _MAXW:
                    imm = [w for w in waits if w.get("wait_mode") == "sem-ge-imm"]
                    rest = [w for w in waits if w.get("wait_mode") != "sem-ge-imm"]
                    n_keep_imm = max(0, _MAXW - len(rest))
                    hoist = imm[: len(imm) - n_keep_imm]
                    keep = rest + imm[len(imm) - n_keep_imm:]
                    if len(keep) > _MAXW:
                        raise RuntimeError(
                            f"instruction {inst.get('name')} has "
                            f"{len(rest)} non-immediate waits; cannot split")
                    for k in range(0, len(hoist), _MAXW):
                        ctr += 1
                        new_insts.append({
                            "debug": inst.get("debug", 0),
                            "engine": inst["engine"],
                            "ins": [], "outs": [],
                            "name": f"WSPL-{ctr}",
                            "opcode": "NoOp",
                            "sync_info": {"on_update": [],
                                          "on_wait": hoist[k:k + _MAXW]},
                            "text_hint": "wait_split",
                        })
                    si["on_wait"] = keep
                    inst["sync_info"] = si
                new_insts.append(inst)
            bb["instructions"] = new_insts
    return bir


_patched = False


def _install_patch():
    global _patched
    if _patched:
        return
    import concourse.bass as bass
    orig = bass.Bass.to_json_bytes

    def patched(self, *a, **kw):
        bir = json.loads(orig(self, *a, **kw))
        return json.dumps(_split_excess_waits(bir)).encode()

    bass.Bass.to_json_bytes = patched
    _patched = True


# --------------------------------------------------------------------------
# Host-side schedule construction
# --------------------------------------------------------------------------
def _round_up(a, b):
    return ((a + b - 1) // b) * b


def _build_layer_schedule(src_tab, dst, sh, blocks):
    """Static SPMD schedule + per-core packed index/dest arrays for one layer.

    Returns (sched, idx16 [NCORES,128,icols_tot], dloc [NCORES,128,nch_tot]).
    """
    n_tiles = math.ceil(sh / P)
    n_groups = math.ceil(n_tiles / GROUP)
    nblk = len(blocks)

    core = dst // sh
    t_loc = (dst % sh) // P
    grp = t_loc // GROUP
    blk = np.zeros(len(src_tab), dtype=np.int64)
    for bi, (base, size) in enumerate(blocks):
        m = (src_tab >= base) & (src_tab < base + size)
        blk[m] = bi
    order = np.lexsort((dst, blk, grp, core))
    src_s, dst_s = src_tab[order], dst[order]
    cell = (core[order] * n_groups + grp[order]) * nblk + blk[order]
    ncells_tot = NCORES * n_groups * nblk
    counts_f = np.bincount(cell, minlength=ncells_tot)
    starts_f = np.concatenate([[0], np.cumsum(counts_f)])[:-1]
    counts = counts_f.reshape(NCORES, n_groups, nblk)
    starts = starts_f.reshape(NCORES, n_groups, nblk)

    n_slots = np.maximum(
        _round_up(counts.max(axis=0), P), P)  # [n_groups, nblk]

    tot_slots = int(n_slots.sum())
    idx16 = np.zeros((NCORES, 16, tot_slots // 16), dtype=np.int16)
    dloc = np.full((NCORES, P, tot_slots // P), OOB, dtype=np.float32)

    sched = []
    icol0 = 0
    chunk0 = 0
    for g in range(n_groups):
        t0 = g * GROUP
        nt = min(GROUP, n_tiles - t0)
        gd = {"t0": t0, "nt": nt, "cells": []}
        for b in range(nblk):
            ns = int(n_slots[g, b])
            nch = ns // P
            base = blocks[b][0]
            spans = [set() for _ in range(nch)]
            for c in range(NCORES):
                cnt = int(counts[c, g, b])
                st = int(starts[c, g, b])
                if cnt:
                    loc = (src_s[st:st + cnt] - base).astype(np.int16)
                    dl = ((dst_s[st:st + cnt] % sh) - t0 * P).astype(np.float32)
                    sl = np.arange(cnt)
                    idx16[c][sl % 16, icol0 + sl // 16] = loc
                    dloc[c][sl % P, chunk0 + sl // P] = dl
                    for k in range(nch):
                        seg = dl[k * P:(k + 1) * P]
                        if len(seg):
                            for t in range(int(seg.min()) // P,
                                           int(seg.max()) // P + 1):
                                spans[k].add(t)
            mm = []
            for k in range(nch):
                tv0 = min(spans[k]) if spans[k] else 0
                if tv0:
                    dloc[:, :, chunk0 + k] -= tv0 * P
                mm.append((k, tv0,
                           [[t, False, False] for t in sorted(spans[k])]))
            gd["cells"].append({"b": b, "icol0": icol0, "icols": ns // 16,
                                "chunk0": chunk0, "nch": nch, "mm": mm})
            icol0 += ns // 16
            chunk0 += nch
        seen = set()
        last_ref = {}
        for cd in gd["cells"]:
            for _k, _tv0, tl in cd["mm"]:
                for ent in tl:
                    if ent[0] not in seen:
                        ent[1] = True
                        seen.add(ent[0])
                    last_ref[ent[0]] = ent
        for ent in last_ref.values():
            ent[2] = True
        sched.append(gd)

    return sched, np.tile(idx16, (1, 8, 1)), dloc


def _mk_blocks(total):
    nb = math.ceil(total / 32768)
    bs = _round_up(math.ceil(total / nb), P)
    while bs > 32768:
        nb += 1
        bs = _round_up(math.ceil(total / nb), P)
    blocks = []
    base = 0
    while base < total:
        blocks.append((base, min(bs, total - base)))
        base += bs
    return blocks


# --------------------------------------------------------------------------
# The kernel
# --------------------------------------------------------------------------
def kernel(x, edge_index, W1, b1, ga, gb, gc, gd, W2, b2):
    _install_patch()
    import concourse.bacc as bacc
    import concourse.mybir as mybir
    import concourse.tile as tile
    from concourse.bass_utils import run_bass_kernel_spmd
    from concourse.masks import make_identity

    x = np.asarray(x, dtype=np.float32)
    edge_index = np.asarray(edge_index)
    W1 = np.asarray(W1, dtype=np.float32)
    b1 = np.asarray(b1, dtype=np.float32)
    W2 = np.asarray(W2, dtype=np.float32)
    b2 = np.asarray(b2, dtype=np.float32)
    ga, gb, gc, gd = float(ga), float(gb), float(gc), float(gd)

    n, in_dim = x.shape
    hid = W1.shape[1]
    out_dim = W2.shape[1]
    assert n % NCORES == 0
    sh = n // NCORES
    n_tiles = math.ceil(sh / P)

    src = np.concatenate([edge_index[0].astype(np.int64),
                          np.arange(n, dtype=np.int64)])
    dst = np.concatenate([edge_index[1].astype(np.int64),
                          np.arange(n, dtype=np.int64)])

    deg = np.bincount(dst, minlength=n).astype(np.float32)
    dinv = np.where(deg > 0, 1.0 / np.sqrt(deg), 0.0).astype(np.float32)

    sh_pad = n_tiles * P
    n_pad1 = sh_pad * NCORES       # g1 rows (padded n)
    n_pad2 = sh_pad * NCORES       # g2 rows (AllGather of padded shards)

    blocks1 = _mk_blocks(n_pad1)
    src1 = (src // sh) * sh_pad + (src % sh)
    # both layers' tables use the same padded row layout -> one schedule
    sched1, idx16_1, dloc1 = _build_layer_schedule(src1, dst, sh, blocks1)

    # GReLU mix: h = z * (k0 + k1*[z<0] + k2*[z>=kc])
    k0 = BETA + (CMIX - BETA) * gb
    k1 = (CMIX - BETA) * (ga - gb)
    k2 = (CMIX - BETA) * (gd - gb)
    kc = gc

    # host-prepared dense arrays (note: g1 table rows use the padded layout,
    # so xT/dinv_full are laid out shard-by-shard with per-shard padding)
    xT = np.zeros((in_dim, n_pad1), dtype=np.float32)
    dinv_pad = np.zeros(n_pad1, dtype=np.float32)
    for c in range(NCORES):
        xT[:, c * sh_pad:c * sh_pad + sh] = x[c * sh:(c + 1) * sh].T
        dinv_pad[c * sh_pad:c * sh_pad + sh] = dinv[c * sh:(c + 1) * sh]
    dinv_full = dinv_pad.reshape(-1, P).T.copy()          # [P, n_pad1/P]
    iota4 = np.tile(np.arange(GROUP * P, dtype=np.float32), (P, 1))
    b1_rep = np.tile(b1, (P, 1)).astype(np.float32)
    b2c_rep = np.tile(CMIX * b2, (P, 1)).astype(np.float32)

    dinv_own = np.zeros((NCORES, P, n_tiles), dtype=np.float32)
    for c in range(NCORES):
        dv = np.zeros(sh_pad, dtype=np.float32)
        dv[:sh] = dinv[c * sh:(c + 1) * sh]
        dinv_own[c] = dv.reshape(n_tiles, P).T
    dinvc_own = (CMIX * dinv_own).astype(np.float32)

    max_icols1 = max(cd["icols"] for g in sched1 for cd in g["cells"])
    max_nch1 = max(cd["nch"] for g in sched1 for cd in g["cells"])

    # ---- build the bass program ----
    nc = bacc.Bacc()
    dt = mybir.dt
    t_xT = nc.dram_tensor("xT", [in_dim, n_pad1], dt.float32, kind="ExternalInput")
    t_W1 = nc.dram_tensor("W1", [in_dim, hid], dt.float32, kind="ExternalInput")
    t_W2 = nc.dram_tensor("W2", [hid, out_dim], dt.float32, kind="ExternalInput")
    t_dinv_full = nc.dram_tensor("dinv_full", [P, n_pad1 // P], dt.float32, kind="ExternalInput")
    t_iota4 = nc.dram_tensor("iota4", [P, GROUP * P], dt.float32, kind="ExternalInput")
    t_b1r = nc.dram_tensor("b1_rep", [P, hid], dt.float32, kind="ExternalInput")
    t_b2r = nc.dram_tensor("b2c_rep", [P, out_dim], dt.float32, kind="ExternalInput")
    t_dinv_own = nc.dram_tensor("dinv_own", [P, n_tiles], dt.float32, kind="ExternalInput")
    t_dinvc_own = nc.dram_tensor("dinvc_own", [P, n_tiles], dt.float32, kind="ExternalInput")
    t_idx1 = nc.dram_tensor("idx16_1", list(idx16_1.shape[1:]), dt.int16, kind="ExternalInput")
    t_dloc1 = nc.dram_tensor("dloc1", list(dloc1.shape[1:]), dt.float32, kind="ExternalInput")
    t_out = nc.dram_tensor("out", [sh, out_dim], dt.float32, kind="ExternalOutput")

    kchunks = in_dim // P
    n_all_tiles = n_pad1 // P
    phases = os.environ.get("GCN_PHASES", "ABCGD")

    with tile.TileContext(nc, pool_alloc_mode="queue") as tc:
        with (
            tc.tile_pool(name="dram", bufs=1, space="DRAM") as dram,
            tc.tile_pool(name="consts", bufs=1) as cpool,
        ):
            g1 = dram.tile([n_pad1, hid], dt.float32)
            g2_in = dram.tile([sh_pad, out_dim], dt.float32)
            g2_full = dram.tile([n_pad2, out_dim], dt.float32)

            w1_t = cpool.tile([P, kchunks, hid], dt.float32)
            for kk in range(kchunks):
                nc.sync.dma_start(out=w1_t[:, kk, :],
                                  in_=t_W1[kk * P:(kk + 1) * P, :])
            w2_t = cpool.tile([hid, out_dim], dt.float32)
            nc.sync.dma_start(out=w2_t[:], in_=t_W2[:])
            dinv_f_t = cpool.tile([P, n_pad1 // P], dt.float32)
            nc.sync.dma_start(out=dinv_f_t[:], in_=t_dinv_full[:])
            iota_t = cpool.tile([P, GROUP * P], dt.float32)
            nc.sync.dma_start(out=iota_t[:], in_=t_iota4[:])
            b1r_t = cpool.tile([P, hid], dt.float32)
            nc.sync.dma_start(out=b1r_t[:], in_=t_b1r[:])
            b2r_t = cpool.tile([P, out_dim], dt.float32)
            nc.sync.dma_start(out=b2r_t[:], in_=t_b2r[:])
            dinv_o_t = cpool.tile([P, n_tiles], dt.float32)
            nc.sync.dma_start(out=dinv_o_t[:], in_=t_dinv_own[:])
            dinvc_o_t = cpool.tile([P, n_tiles], dt.float32)
            nc.sync.dma_start(out=dinvc_o_t[:], in_=t_dinvc_own[:])
            dloc1_t = cpool.tile([P, dloc1.shape[2]], dt.float32)
            nc.sync.dma_start(out=dloc1_t[:], in_=t_dloc1[:])
            ident_t = cpool.tile([P, P], dt.float32)
            make_identity(nc, ident_t[:])

            # ---- phase A: dense L1 (replicated) ----
            SUP = 4
            if "A" in phases:
              with (
                tc.tile_pool(name="xin", bufs=3) as xpool,
                tc.tile_pool(name="dense_ps", bufs=8, space="PSUM") as dps,
                tc.tile_pool(name="dense_out", bufs=4) as dout,
              ):
                for st in range(math.ceil(n_all_tiles / SUP)):
                    j0 = st * SUP
                    jn = min(SUP, n_all_tiles - j0)
                    xk = xpool.tile([P, kchunks, SUP * P], dt.float32, tag="xk")
                    for kk in range(kchunks):
                        nc.sync.dma_start(
                            out=xk[:, kk, :jn * P],
                            in_=t_xT[kk * P:(kk + 1) * P, j0 * P:(j0 + jn) * P])
                    for j in range(jn):
                        t_glob = j0 + j
                        ps = dps.tile([P, hid], dt.float32, space="PSUM",
                                      tag="dps")
                        for kk in range(kchunks):
                            nc.tensor.matmul(
                                out=ps[:],
                                lhsT=xk[:, kk, j * P:(j + 1) * P],
                                rhs=w1_t[:, kk, :],
                                start=(kk == 0), stop=(kk == kchunks - 1))
                        gt = dout.tile([P, hid], dt.float32, tag="gout")
                        nc.vector.tensor_scalar(
                            out=gt[:], in0=ps[:],
                            scalar1=dinv_f_t[:, t_glob:t_glob + 1],
                            scalar2=None, op0=mybir.AluOpType.mult)
                        nc.sync.dma_start(
                            out=g1[t_glob * P:(t_glob + 1) * P, :], in_=gt[:])

            # ---- shared gather + segment-sum aggregation ----
            def aggregate(sched, dloc_t, t_idx, max_icols, max_nch, table,
                          elem, blocks, epilogue, gat, gidx, smat, sps, tagp):
                for gd in sched:
                    psums = {}
                    for cd in gd["cells"]:
                        ns = cd["nch"] * P
                        nch = cd["nch"]
                        it = gidx.tile([P, max_icols], dt.int16, tag="idx")
                        nc.sync.dma_start(
                            out=it[:, :cd["icols"]],
                            in_=t_idx[:, cd["icol0"]:cd["icol0"] + cd["icols"]])
                        gt = gat.tile([P, max_nch, elem], dt.float32, tag="g")
                        base, bsize = blocks[cd["b"]]
                        nc.gpsimd.dma_gather(
                            out_ap=gt[:, :cd["nch"], :],
                            in_ap=table[base:base + bsize, :],
                            idxs_ap=it[:, :cd["icols"]],
                            num_idxs=ns, num_idxs_reg=ns, elem_size=elem,
                            single_packet=False)
                        # matmul list per chunk, with per-(chunk,tile) iota
                        # variants; build ALL the one-hot matrices of this
                        # cell in one DVE op per iota variant span.
                        # S[p, k, m] = (iota[m + 128*tv(k)] == dloc[p, ch(k)])
                        # We emit one op per chunk-span variant layout by
                        # building the full [P, nch, P] tensor per variant
                        # actually needed; simpler: one op builds variant 0
                        # comparisons for all chunks, and span matmuls with
                        # tv>0 use extra per-chunk ops (rare).
                        s_cell = smat.tile([P, max_nch, P], dt.float32,
                                           tag="s")
                        c0 = cd["chunk0"]
                        nc.vector.tensor_tensor(
                            out=s_cell[:, :nch, :],
                            in0=iota_t[:, :P].rearrange("p (c m) -> p c m", c=1)
                                .to_broadcast([P, nch, P]),
                            in1=dloc_t[:, c0:c0 + nch]
                                .rearrange("p (c o) -> p c o", o=1)
                                .to_broadcast([P, nch, P]),
                            op=mybir.AluOpType.is_equal)
                        for kloc, tv0, tlist in cd["mm"]:
                            ch = cd["chunk0"] + kloc
                            for (tv, first, last) in tlist:
                                if tv not in psums:
                                    pst_new = sps.tile(
                                        [P, elem], dt.float32, space="PSUM",
                                        tag=tagp, name=f"ps_{tagp}_{tv}")
                                    psums[tv] = pst_new
                                rel = tv - tv0
                                if rel == 0:
                                    s_ap = s_cell[:, kloc, :]
                                else:
                                    s_x = smat.tile([P, P], dt.float32,
                                                    tag="sx")
                                    nc.vector.tensor_scalar(
                                        out=s_x[:],
                                        in0=iota_t[:, rel * P:(rel + 1) * P],
                                        scalar1=dloc_t[:, ch:ch + 1],
                                        scalar2=None,
                                        op0=mybir.AluOpType.is_equal)
                                    s_ap = s_x[:]
                                nc.tensor.matmul(
                                    out=psums[tv][:], lhsT=s_ap,
                                    rhs=gt[:, kloc, :],
                                    start=first, stop=last)
                    for tv in sorted(psums):
                        epilogue(gd["t0"] + tv, psums[tv])

            # ---- phase B: L1 aggregation + GReLU mix ----
            if "B" in phases:
              with (
                tc.tile_pool(name="gat1", bufs=3) as gat,
                tc.tile_pool(name="gidx1", bufs=3) as gidx,
                tc.tile_pool(name="smat1", bufs=4) as smat,
                tc.tile_pool(name="seg_ps1", bufs=6, space="PSUM") as sps,
                tc.tile_pool(name="c_ps", bufs=1, space="PSUM") as cps,
                tc.tile_pool(name="c_ps2", bufs=1, space="PSUM") as cps2,
                tc.tile_pool(name="epi1", bufs=4) as epool,
              ):
                def epi1(t, ps):
                    z = epool.tile([P, hid], dt.float32, tag="z")
                    nc.vector.tensor_scalar(
                        out=z[:], in0=ps[:], scalar1=dinv_o_t[:, t:t + 1],
                        scalar2=None, op0=mybir.AluOpType.mult)
                    nc.vector.tensor_tensor(
                        out=z[:], in0=z[:], in1=b1r_t[:],
                        op=mybir.AluOpType.add)
                    mneg = epool.tile([P, hid], dt.float32, tag="mneg")
                    nc.vector.tensor_scalar(
                        out=mneg[:], in0=z[:], scalar1=0.0, scalar2=None,
                        op0=mybir.AluOpType.is_lt)
                    mhi = epool.tile([P, hid], dt.float32, tag="mhi")
                    nc.vector.tensor_scalar(
                        out=mhi[:], in0=z[:], scalar1=kc, scalar2=None,
                        op0=mybir.AluOpType.is_ge)
                    coef = epool.tile([P, hid], dt.float32, tag="coef")
                    nc.vector.tensor_scalar(
                        out=coef[:], in0=mneg[:], scalar1=k1, scalar2=k0,
                        op0=mybir.AluOpType.mult, op1=mybir.AluOpType.add)
                    nc.vector.tensor_scalar(
                        out=mhi[:], in0=mhi[:], scalar1=k2, scalar2=None,
                        op0=mybir.AluOpType.mult)
                    nc.vector.tensor_tensor(
                        out=coef[:], in0=coef[:], in1=mhi[:],
                        op=mybir.AluOpType.add)
                    nc.vector.tensor_tensor(
                        out=z[:], in0=z[:], in1=coef[:],
                        op=mybir.AluOpType.mult)
                    # fused dense L2 for this tile: g2 = dinv * (h @ W2)
                    pst = cps.tile([P, P], dt.float32, space="PSUM",
                                   tag="tps")
                    nc.tensor.transpose(out=pst[:], in_=z[:],
                                        identity=ident_t[:])
                    hT = epool.tile([P, P], dt.float32, tag="hT")
                    nc.vector.tensor_copy(out=hT[:], in_=pst[:])
                    ps2 = cps2.tile([P, out_dim], dt.float32, space="PSUM",
                                    tag="d2ps")
                    nc.tensor.matmul(out=ps2[:], lhsT=hT[:], rhs=w2_t[:],
                                     start=True, stop=True)
                    g2t = epool.tile([P, out_dim], dt.float32, tag="g2out")
                    nc.vector.tensor_scalar(
                        out=g2t[:], in0=ps2[:],
                        scalar1=dinv_o_t[:, t:t + 1], scalar2=None,
                        op0=mybir.AluOpType.mult)
                    nc.sync.dma_start(out=g2_in[t * P:(t + 1) * P, :],
                                      in_=g2t[:])

                aggregate(sched1, dloc1_t, t_idx1, max_icols1, max_nch1,
                          g1[:], hid, blocks1, epi1, gat, gidx, smat, sps,
                          "segps1")

            # (phase C fused into the B epilogue above)

            if "G" in phases:
                nc.gpsimd.collective_compute(
                    "AllGather", mybir.AluOpType.bypass,
                    replica_groups=[list(range(NCORES))],
                    ins=[g2_in[:].opt()], outs=[g2_full[:].opt()])

            # ---- phase D: L2 aggregation + output ----
            if "D" in phases:
              with (
                tc.tile_pool(name="gat2", bufs=3) as gat,
                tc.tile_pool(name="gidx2", bufs=3) as gidx,
                tc.tile_pool(name="smat2", bufs=4) as smat,
                tc.tile_pool(name="seg_ps2", bufs=2 * GROUP, space="PSUM") as sps,
                tc.tile_pool(name="epi2", bufs=4) as epool,
              ):
                def epi2(t, ps):
                    z = epool.tile([P, out_dim], dt.float32, tag="z2")
                    nc.vector.tensor_scalar(
                        out=z[:], in0=ps[:], scalar1=dinvc_o_t[:, t:t + 1],
                        scalar2=None, op0=mybir.AluOpType.mult)
                    nc.vector.tensor_tensor(
                        out=z[:], in0=z[:], in1=b2r_t[:],
                        op=mybir.AluOpType.add)
                    lo = t * P
                    hi = min((t + 1) * P, sh)
                    nc.sync.dma_start(out=t_out[lo:hi, :], in_=z[:hi - lo, :])

                aggregate(sched1, dloc1_t, t_idx1, max_icols1, max_nch1,
                          g2_full[:], out_dim, blocks1, epi2, gat, gidx,
                          smat, sps, "segps2")

    nc.finalize()

    in_maps = []
    for c in range(NCORES):
        in_maps.append({
            "xT": xT, "W1": W1, "W2": W2,
            "dinv_full": dinv_full, "iota4": iota4,
            "b1_rep": b1_rep, "b2c_rep": b2c_rep,
            "dinv_own": dinv_own[c], "dinvc_own": dinvc_own[c],
            "idx16_1": idx16_1[c], "dloc1": dloc1[c],
        })

    import time
    t0 = time.monotonic()
    r = run_bass_kernel_spmd(nc, in_maps, list(range(NCORES)))
    _EXEC_STATS["first_call_s"] = time.monotonic() - t0
    _EXEC_STATS["results"] = r
    _EXEC_STATS["nc"] = nc
    _EXEC_STATS["in_maps"] = in_maps

    return np.concatenate([r.results[c]["out"] for c in range(NCORES)],
                          axis=0)



# revision 20
# speedup vs baseline: 2.5795x; 2.5795x over previous
"""MixGCN (2-layer GCN with GReLU mix) on 8 Trainium2 NeuronCores.

Sharding/dataflow (v2):
  L1 (aggregate-first, per-dest-core): z1 = (A~ x) W1. Each core gathers
      dinv-prescaled x rows (fp16, 512B descriptors) for the edges whose
      DESTINATION lies in its node shard, segment-sums them via one-hot
      matmuls into PSUM *transposed* ([k, dest]), then applies W1 with a
      second matmul pair (no transposes anywhere), the GReLU mix epilogue
      on z1^T, and the fused L2 transform g2 = dinv * (h @ W2) for its own
      rows -> local table g2_in.
  L2 (scatter + ReduceScatter): each core aggregates its OWN-SOURCE edges
      toward ALL destinations (one-hot matmuls over gathered g2_in rows),
      writes partial sums [n_pad, 64] fp16, then a ReduceScatter(add)
      hands each core its own destination shard; a tiny epilogue applies
      dinv[dst] and bias.
All loop bounds are static and identical across cores (SPMD); slot counts
per cell are the max over cores, padded slots gather row 0 with an
out-of-range dest id so their one-hot rows are all-zero.
"""
import json
import math
import os

import numpy as np

P = 128
NCORES = 8
import os as _os
GROUP1 = int(_os.environ.get("GCN_G1", "4"))   # dest tiles per L1 group
GROUP2 = int(_os.environ.get("GCN_G2", "8"))   # dest tiles per L2 group
OOB = 30000.0      # dest-local sentinel for padded slots (fp16-exact)
BETA, CMIX = 0.5, 1.0

_EXEC_STATS = {}   # the test harness reads timing info from here


# --------------------------------------------------------------------------
# Workaround for this walrus build's per-instruction sync-wait limit: hoist
# excess immediate semaphore waits onto NoOps inserted before the offending
# instruction on the same engine (sem values are monotonic in-kernel, so
# waiting earlier is equivalent).
# --------------------------------------------------------------------------
_MAXW = 1


def _split_excess_waits(bir: dict) -> dict:
    ctr = 0
    for fn in bir.get("functions", []):
        for bb in fn.get("blocks", []):
            insts = bb.get("instructions", [])
            if not any(
                len(((i.get("sync_info") or {}).get("on_wait") or [])) > _MAXW
                for i in insts
            ):
                continue
            new_insts = []
            for inst in insts:
                si = inst.get("sync_info") or {}
                waits = si.get("on_wait") or []
                if len(waits) > _MAXW:
                    imm = [w for w in waits if w.get("wait_mode") == "sem-ge-imm"]
                    rest = [w for w in waits if w.get("wait_mode") != "sem-ge-imm"]
                    n_keep_imm = max(0, _MAXW - len(rest))
                    hoist = imm[: len(imm) - n_keep_imm]
                    keep = rest + imm[len(imm) - n_keep_imm:]
                    if len(keep) > _MAXW:
                        raise RuntimeError(
                            f"instruction {inst.get('name')} has "
                            f"{len(rest)} non-immediate waits; cannot split")
                    for k in range(0, len(hoist), _MAXW):
                        ctr += 1
                        new_insts.append({
                            "debug": inst.get("debug", 0),
                            "engine": inst["engine"],
                            "ins": [], "outs": [],
                            "name": f"WSPL-{ctr}",
                            "opcode": "NoOp",
                            "sync_info": {"on_update": [],
                                          "on_wait": hoist[k:k + _MAXW]},
                            "text_hint": "wait_split",
                        })
                    si["on_wait"] = keep
                    inst["sync_info"] = si
                new_insts.append(inst)
            bb["instructions"] = new_insts
    return bir


_patched = False


def _install_patch():
    global _patched
    if _patched:
        return
    import concourse.bass as bass
    orig = bass.Bass.to_json_bytes

    def patched(self, *a, **kw):
        bir = json.loads(orig(self, *a, **kw))
        return json.dumps(_split_excess_waits(bir)).encode()

    bass.Bass.to_json_bytes = patched
    _patched = True


# --------------------------------------------------------------------------
# Host-side schedule construction
# --------------------------------------------------------------------------
def _round_up(a, b):
    return ((a + b - 1) // b) * b


def _build_sched(core, row, dst_t, n_tiles, group, blocks):
    """Static SPMD schedule + per-core packed index/dest arrays.

    core[e]:  core that processes edge e
    row[e]:   table row gathered for edge e (global over `blocks`)
    dst_t[e]: destination offset in this layer's padded output space

    Returns (sched, idx16 [NCORES,128,icols_tot], dloc [NCORES,128,nch_tot]).
    """
    n_groups = math.ceil(n_tiles / group)
    nblk = len(blocks)

    grp = (dst_t // P) // group
    blk = np.zeros(len(row), dtype=np.int64)
    for bi, (base, size) in enumerate(blocks):
        m = (row >= base) & (row < base + size)
        blk[m] = bi
    order = np.lexsort((dst_t, blk, grp, core))
    row_s, dst_s = row[order], dst_t[order]
    cell = (core[order] * n_groups + grp[order]) * nblk + blk[order]
    ncells_tot = NCORES * n_groups * nblk
    counts_f = np.bincount(cell, minlength=ncells_tot)
    starts_f = np.concatenate([[0], np.cumsum(counts_f)])[:-1]
    counts = counts_f.reshape(NCORES, n_groups, nblk)
    starts = starts_f.reshape(NCORES, n_groups, nblk)

    n_slots = np.maximum(
        _round_up(counts.max(axis=0), P), P)  # [n_groups, nblk]

    tot_slots = int(n_slots.sum())
    idx16 = np.zeros((NCORES, 16, tot_slots // 16), dtype=np.int16)
    dloc = np.full((NCORES, P, tot_slots // P), OOB, dtype=np.float32)

    sched = []
    icol0 = 0
    chunk0 = 0
    for g in range(n_groups):
        t0 = g * group
        nt = min(group, n_tiles - t0)
        gd = {"t0": t0, "nt": nt, "cells": []}
        for b in range(nblk):
            ns = int(n_slots[g, b])
            nch = ns // P
            base = blocks[b][0]
            spans = [set() for _ in range(nch)]
            for c in range(NCORES):
                cnt = int(counts[c, g, b])
                st = int(starts[c, g, b])
                if cnt:
                    loc = (row_s[st:st + cnt] - base).astype(np.int16)
                    dl = (dst_s[st:st + cnt] - t0 * P).astype(np.float32)
                    sl = np.arange(cnt)
                    idx16[c][sl % 16, icol0 + sl // 16] = loc
                    dloc[c][sl % P, chunk0 + sl // P] = dl
                    dlf = dl.astype(np.int64)
                    for k in range(nch):
                        seg = dlf[k * P:(k + 1) * P]
                        if len(seg):
                            for t in range(int(seg.min()) // P,
                                           int(seg.max()) // P + 1):
                                spans[k].add(t)
            mm = []
            for k in range(nch):
                tv0 = min(spans[k]) if spans[k] else 0
                if tv0:
                    dloc[:, :, chunk0 + k] -= np.float32(tv0 * P)
                mm.append((k, tv0,
                           [[t, False, False] for t in sorted(spans[k])]))
            gd["cells"].append({"b": b, "icol0": icol0, "icols": ns // 16,
                                "chunk0": chunk0, "nch": nch, "mm": mm})
            icol0 += ns // 16
            chunk0 += nch
        seen = set()
        last_ref = {}
        for cd in gd["cells"]:
            for _k, _tv0, tl in cd["mm"]:
                for ent in tl:
                    if ent[0] not in seen:
                        ent[1] = True
                        seen.add(ent[0])
                    last_ref[ent[0]] = ent
        for ent in last_ref.values():
            ent[2] = True
        sched.append(gd)

    return sched, np.tile(idx16, (1, 8, 1)), dloc


def _mk_blocks(total):
    nb = math.ceil(total / 32768)
    bs = _round_up(math.ceil(total / nb), P)
    while bs > 32768:
        nb += 1
        bs = _round_up(math.ceil(total / nb), P)
    blocks = []
    base = 0
    while base < total:
        blocks.append((base, min(bs, total - base)))
        base += bs
    return blocks


# --------------------------------------------------------------------------
# The kernel
# --------------------------------------------------------------------------
def kernel(x, edge_index, W1, b1, ga, gb, gc, gd, W2, b2):
    _install_patch()
    import concourse.bacc as bacc
    import concourse.mybir as mybir
    import concourse.tile as tile
    from concourse.bass_utils import run_bass_kernel_spmd
    from concourse.masks import make_identity

    x = np.asarray(x, dtype=np.float32)
    edge_index = np.asarray(edge_index)
    W1 = np.asarray(W1, dtype=np.float32)
    b1 = np.asarray(b1, dtype=np.float32)
    W2 = np.asarray(W2, dtype=np.float32)
    b2 = np.asarray(b2, dtype=np.float32)
    ga, gb, gc, gd = float(ga), float(gb), float(gc), float(gd)

    n, in_dim = x.shape
    hid = W1.shape[1]
    out_dim = W2.shape[1]
    assert n % NCORES == 0
    sh = n // NCORES
    n_tiles = math.ceil(sh / P)          # own-shard dest tiles (98)
    sh_pad = n_tiles * P                 # 12544
    n_pad = sh_pad * NCORES              # padded global rows (100352)
    n_tiles_g = n_pad // P               # global dest tiles (784)
    kchunks = in_dim // P                # 2

    src = np.concatenate([edge_index[0].astype(np.int64),
                          np.arange(n, dtype=np.int64)])
    dst = np.concatenate([edge_index[1].astype(np.int64),
                          np.arange(n, dtype=np.int64)])

    deg = np.bincount(dst, minlength=n).astype(np.float32)
    dinv = np.where(deg > 0, 1.0 / np.sqrt(deg), 0.0).astype(np.float32)

    # ---- L1 schedule: by destination core; table = xs (padded to n_rows1)
    n_rows1 = _round_up(n, P)
    blocks1 = _mk_blocks(n_rows1)
    sched1, idx16_1, dloc1 = _build_sched(
        dst // sh, src, dst % sh, n_tiles, GROUP1, blocks1)

    # ---- L2 schedule: by source core; table = own g2_in (sh_pad rows);
    #      dest space = padded global rows
    dst_pad = (dst // sh) * sh_pad + (dst % sh)
    sched2, idx16_2, dloc2 = _build_sched(
        src // sh, src % sh, dst_pad, n_tiles_g, GROUP2, [(0, sh_pad)])

    # GReLU mix: h = z * (k0 + k1*[z<0] + k2*[z>=kc])
    k0 = BETA + (CMIX - BETA) * gb
    k1 = (CMIX - BETA) * (ga - gb)
    k2 = (CMIX - BETA) * (gd - gb)
    kc = gc

    # ---- host-prepared dense arrays
    xs = np.zeros((n_rows1, in_dim), dtype=np.float16)
    xs[:n] = (dinv[:, None] * x).astype(np.float16)
    w1c = np.ascontiguousarray(
        W1.reshape(kchunks, P, hid).transpose(1, 0, 2)).astype(np.float16)
    w2h = W2.astype(np.float16)                          # [hid, out]
    iota = np.tile(np.arange(GROUP2 * P, dtype=np.float16), (P, 1))
    b1col = b1.reshape(hid, 1).astype(np.float32)
    b2r = np.tile(CMIX * b2, (P, 1)).astype(np.float32)

    dinvT_own = np.zeros((NCORES, P, sh_pad), dtype=np.float16)
    dinv_own = np.zeros((NCORES, P, n_tiles), dtype=np.float32)
    for c in range(NCORES):
        dv = np.zeros(sh_pad, dtype=np.float32)
        dv[:sh] = dinv[c * sh:(c + 1) * sh]
        dinvT_own[c] = np.tile(dv.astype(np.float16), (P, 1))
        dinv_own[c] = dv.reshape(n_tiles, P).T
    dinvc_own = (CMIX * dinv_own).astype(np.float32)

    max_icols1 = max(cd["icols"] for g in sched1 for cd in g["cells"])
    max_nch1 = max(cd["nch"] for g in sched1 for cd in g["cells"])
    max_icols2 = max(cd["icols"] for g in sched2 for cd in g["cells"])
    max_nch2 = max(cd["nch"] for g in sched2 for cd in g["cells"])

    # ---- build the bass program ----
    nc = bacc.Bacc()
    dt = mybir.dt
    t_xs = nc.dram_tensor("xs", [n_rows1, in_dim], dt.float16,
                          kind="ExternalInput")
    t_w1c = nc.dram_tensor("w1c", [P, kchunks, hid], dt.float16,
                           kind="ExternalInput")
    t_w2h = nc.dram_tensor("w2h", [hid, out_dim], dt.float16,
                           kind="ExternalInput")
    t_iota = nc.dram_tensor("iota", [P, GROUP2 * P], dt.float16,
                            kind="ExternalInput")
    t_b1col = nc.dram_tensor("b1col", [hid, 1], dt.float32,
                             kind="ExternalInput")
    t_b2r = nc.dram_tensor("b2r", [P, out_dim], dt.float32,
                           kind="ExternalInput")
    t_dinvT = nc.dram_tensor("dinvT", [P, sh_pad], dt.float16,
                             kind="ExternalInput")
    t_dinv_own = nc.dram_tensor("dinv_own", [P, n_tiles], dt.float32,
                                kind="ExternalInput")
    t_dinvc_own = nc.dram_tensor("dinvc_own", [P, n_tiles], dt.float32,
                                 kind="ExternalInput")
    t_idx1 = nc.dram_tensor("idx1", list(idx16_1.shape[1:]), dt.int16,
                            kind="ExternalInput")
    t_dloc1 = nc.dram_tensor("dloc1", list(dloc1.shape[1:]), dt.float32,
                             kind="ExternalInput")
    t_idx2 = nc.dram_tensor("idx2", list(idx16_2.shape[1:]), dt.int16,
                            kind="ExternalInput")
    t_dloc2 = nc.dram_tensor("dloc2", list(dloc2.shape[1:]), dt.float32,
                             kind="ExternalInput")
    t_out = nc.dram_tensor("out", [sh, out_dim], dt.float32,
                           kind="ExternalOutput")
    debug = bool(os.environ.get("GCN_DEBUG"))
    if debug:
        t_dbg_g2 = nc.dram_tensor("dbg_g2", [sh_pad, P], dt.float16,
                                  kind="ExternalOutput")
        t_dbg_pt = nc.dram_tensor("dbg_pt", [n_pad, out_dim], dt.float16,
                                  kind="ExternalOutput")
        t_dbg_rs = nc.dram_tensor("dbg_rs", [sh_pad, out_dim], dt.float16,
                                  kind="ExternalOutput")
        t_dbg_ax = nc.dram_tensor("dbg_ax", [sh_pad, in_dim], dt.float16,
                                  kind="ExternalOutput")
        t_dbg_z = nc.dram_tensor("dbg_z", [sh_pad, hid], dt.float16,
                                 kind="ExternalOutput")

    phases = os.environ.get("GCN_PHASES", "BDRF")

    with tile.TileContext(nc, pool_alloc_mode="queue") as tc:
        with (
            tc.tile_pool(name="dram", bufs=1, space="DRAM") as dram,
            tc.tile_pool(name="consts", bufs=1) as cpool,
        ):
            g2_in = dram.tile([sh_pad, P], dt.float16)   # cols 64: junk
            partial = dram.tile([n_pad, out_dim], dt.float16)
            rs_out = dram.tile([sh_pad, out_dim], dt.float16)

            w1_t = cpool.tile([P, kchunks, hid], dt.float16)
            nc.sync.dma_start(out=w1_t[:], in_=t_w1c[:])
            w2_t = cpool.tile([hid, out_dim], dt.float16)
            nc.sync.dma_start(out=w2_t[:], in_=t_w2h[:])
            iota_t = cpool.tile([P, GROUP2 * P], dt.float16)
            nc.sync.dma_start(out=iota_t[:], in_=t_iota[:])
            b1c_t = cpool.tile([hid, 1], dt.float32)
            nc.sync.dma_start(out=b1c_t[:], in_=t_b1col[:])
            b2r_t = cpool.tile([P, out_dim], dt.float32)
            nc.sync.dma_start(out=b2r_t[:], in_=t_b2r[:])
            dinvT_t = cpool.tile([P, sh_pad], dt.float16)
            nc.sync.dma_start(out=dinvT_t[:], in_=t_dinvT[:])
            dinv_o_t = cpool.tile([P, n_tiles], dt.float32)
            nc.sync.dma_start(out=dinv_o_t[:], in_=t_dinv_own[:])
            dinvc_o_t = cpool.tile([P, n_tiles], dt.float32)
            nc.sync.dma_start(out=dinvc_o_t[:], in_=t_dinvc_own[:])
            dloc1_t = cpool.tile([P, dloc1.shape[2]], dt.float32)
            nc.sync.dma_start(out=dloc1_t[:], in_=t_dloc1[:])
            dloc2_t = cpool.tile([P, dloc2.shape[2]], dt.float32)
            nc.sync.dma_start(out=dloc2_t[:], in_=t_dloc2[:])
            ident_t = cpool.tile([P, P], dt.float16)
            make_identity(nc, ident_t[:])

            # ---- phase B: L1 gather + transposed segment-sum + dense +
            #      GReLU mix + fused L2 transform ----
            if "B" in phases:
              with (
                tc.tile_pool(name="gat1", bufs=3) as gat,
                tc.tile_pool(name="gidx1", bufs=3) as gidx,
                tc.tile_pool(name="smat1", bufs=8) as smat,
                tc.tile_pool(name="seg_ps1", bufs=GROUP1 + 1,
                             space="PSUM") as sps,
                tc.tile_pool(name="z_ps", bufs=2, space="PSUM") as zps,
                tc.tile_pool(name="t_ps", bufs=1, space="PSUM") as tps,
                tc.tile_pool(name="epi1", bufs=10) as epool,
              ):
                for gd_ in sched1:
                    psums = {}
                    for cd in gd_["cells"]:
                        ns = cd["nch"] * P
                        nch = cd["nch"]
                        it = gidx.tile([P, max_icols1], dt.int16, tag="idx")
                        nc.sync.dma_start(
                            out=it[:, :cd["icols"]],
                            in_=t_idx1[:, cd["icol0"]:cd["icol0"] + cd["icols"]])
                        gt = gat.tile([P, max_nch1, in_dim], dt.float16,
                                      tag="g")
                        base, bsize = blocks1[cd["b"]]
                        nc.gpsimd.dma_gather(
                            out_ap=gt[:, :nch, :],
                            in_ap=t_xs[base:base + bsize, :],
                            idxs_ap=it[:, :cd["icols"]],
                            num_idxs=ns, num_idxs_reg=ns, elem_size=in_dim,
                            single_packet=False)
                        for kloc, tv0, tlist in cd["mm"]:
                            ch = cd["chunk0"] + kloc
                            for (tv, first, last) in tlist:
                                if tv not in psums:
                                    psums[tv] = sps.tile(
                                        [P, in_dim], dt.float32,
                                        space="PSUM", tag="segps1",
                                        name=f"ps1_{tv}")
                                rel = tv - tv0
                                s_x = smat.tile([P, P], dt.float16, tag="sx")
                                nc.vector.tensor_scalar(
                                    out=s_x[:],
                                    in0=iota_t[:, rel * P:(rel + 1) * P],
                                    scalar1=dloc1_t[:, ch:ch + 1],
                                    scalar2=None,
                                    op0=mybir.AluOpType.is_equal)
                                nc.tensor.matmul(
                                    out=psums[tv][:],
                                    lhsT=s_x[:],
                                    rhs=gt[:, kloc, :],
                                    start=first, stop=last)
                    touched = sorted(
                        tv for _cd in gd_["cells"]
                        for _k, _tv0, _tl in _cd["mm"] for (tv, _f, _l) in _tl)
                    for tv in sorted(set(touched)):
                        t = gd_["t0"] + tv
                        if t >= n_tiles:
                            continue
                        ax = epool.tile([P, in_dim], dt.float16, tag="ax")
                        nc.scalar.copy(out=ax[:], in_=psums[tv][:])
                        axT = epool.tile([P, kchunks, P], dt.float16,
                                         tag="axT")
                        for kk in range(kchunks):
                            tp = tps.tile([P, P], dt.float16, space="PSUM",
                                          tag="tps")
                            nc.tensor.transpose(
                                out=tp[:], in_=ax[:, kk * P:(kk + 1) * P],
                                identity=ident_t[:])
                            nc.scalar.copy(out=axT[:, kk, :], in_=tp[:])
                        zb = zps.tile([P, P + out_dim], dt.float32,
                                      space="PSUM", tag="zps")
                        zp = zb[:, :P]
                        for kk in range(kchunks):
                            nc.tensor.matmul(
                                out=zp, lhsT=w1_t[:, kk, :],
                                rhs=axT[:, kk, :],
                                start=(kk == 0), stop=(kk == kchunks - 1))
                        # epilogue on z1^T [h, dest]
                        zt = epool.tile([P, P], dt.float16, tag="zt")
                        nc.vector.tensor_tensor(
                            out=zt[:], in0=zp,
                            in1=dinvT_t[:, t * P:(t + 1) * P],
                            op=mybir.AluOpType.mult)
                        nc.vector.tensor_scalar(
                            out=zt[:], in0=zt[:], scalar1=b1c_t[:, 0:1],
                            scalar2=None, op0=mybir.AluOpType.add)
                        if debug:
                            nc.sync.dma_start(
                                out=t_dbg_ax[t * P:(t + 1) * P, :],
                                in_=ax[:])
                            nc.sync.dma_start(
                                out=t_dbg_z[t * P:(t + 1) * P, :],
                                in_=zt[:])
                        mn = epool.tile([P, P], dt.float16, tag="mn")
                        nc.vector.tensor_scalar(
                            out=mn[:], in0=zt[:], scalar1=0.0, scalar2=None,
                            op0=mybir.AluOpType.is_lt)
                        mh = epool.tile([P, P], dt.float16, tag="mh")
                        nc.vector.tensor_scalar(
                            out=mh[:], in0=zt[:], scalar1=kc, scalar2=None,
                            op0=mybir.AluOpType.is_ge)
                        nc.vector.tensor_scalar(
                            out=mn[:], in0=mn[:], scalar1=k1, scalar2=k0,
                            op0=mybir.AluOpType.mult,
                            op1=mybir.AluOpType.add)
                        nc.vector.tensor_scalar(
                            out=mh[:], in0=mh[:], scalar1=k2, scalar2=None,
                            op0=mybir.AluOpType.mult)
                        nc.vector.tensor_tensor(
                            out=mn[:], in0=mn[:], in1=mh[:],
                            op=mybir.AluOpType.add)
                        ht = epool.tile([P, P], dt.float16, tag="ht")
                        nc.vector.tensor_tensor(
                            out=ht[:], in0=zt[:], in1=mn[:],
                            op=mybir.AluOpType.mult)
                        # fused L2 transform: g2 = dinv * (h @ W2)
                        gp = zb[:, P:P + out_dim]
                        nc.tensor.matmul(out=gp, lhsT=ht[:], rhs=w2_t[:],
                                         start=True, stop=True)
                        g2t = epool.tile([P, out_dim], dt.float16, tag="g2t")
                        nc.vector.tensor_scalar(
                            out=g2t[:], in0=gp,
                            scalar1=dinv_o_t[:, t:t + 1], scalar2=None,
                            op0=mybir.AluOpType.mult)
                        nc.sync.dma_start(
                            out=g2_in[t * P:(t + 1) * P, :out_dim],
                            in_=g2t[:])

            # ---- phase D: L2 scatter partial sums over ALL destinations ----
            if "D" in phases:
              with (
                tc.tile_pool(name="gat2", bufs=3) as gat,
                tc.tile_pool(name="gidx2", bufs=3) as gidx,
                tc.tile_pool(name="smat2", bufs=8) as smat,
                tc.tile_pool(name="seg_ps2", bufs=GROUP2,
                             space="PSUM") as sps,
                tc.tile_pool(name="stage2", bufs=3) as stg,
              ):
                for gi, gd_ in enumerate(sched2):
                    sg = stg.tile([P, GROUP2, out_dim], dt.float16, tag="sg")
                    psums = {}
                    for cd in gd_["cells"]:
                        ns = cd["nch"] * P
                        nch = cd["nch"]
                        it = gidx.tile([P, max_icols2], dt.int16, tag="idx")
                        nc.sync.dma_start(
                            out=it[:, :cd["icols"]],
                            in_=t_idx2[:, cd["icol0"]:cd["icol0"] + cd["icols"]])
                        gt = gat.tile([P, max_nch2, P], dt.float16, tag="g")
                        nc.gpsimd.dma_gather(
                            out_ap=gt[:, :nch, :],
                            in_ap=g2_in[:, :],
                            idxs_ap=it[:, :cd["icols"]],
                            num_idxs=ns, num_idxs_reg=ns, elem_size=P,
                            single_packet=False)
                        for kloc, tv0, tlist in cd["mm"]:
                            ch = cd["chunk0"] + kloc
                            for (tv, first, last) in tlist:
                                if tv not in psums:
                                    psums[tv] = sps.tile(
                                        [P, out_dim], dt.float32,
                                        space="PSUM", tag="segps2",
                                        name=f"ps2_{tv}")
                                rel = tv - tv0
                                s_x = smat.tile([P, P], dt.float16, tag="sx")
                                nc.vector.tensor_scalar(
                                    out=s_x[:],
                                    in0=iota_t[:, rel * P:(rel + 1) * P],
                                    scalar1=dloc2_t[:, ch:ch + 1],
                                    scalar2=None,
                                    op0=mybir.AluOpType.is_equal)
                                nc.tensor.matmul(
                                    out=psums[tv][:],
                                    lhsT=s_x[:],
                                    rhs=gt[:, kloc, :out_dim],
                                    start=first, stop=last)
                    touched = set(
                        tv for _cd in gd_["cells"]
                        for _k, _tv0, _tl in _cd["mm"] for (tv, _f, _l) in _tl)
                    for tv in range(gd_["nt"]):
                        if tv in touched:
                            nc.scalar.copy(out=sg[:, tv, :],
                                           in_=psums[tv][:])
                        else:
                            nc.scalar.memzero(sg[:, tv, :])
                    r0 = gd_["t0"] * P
                    rows = gd_["nt"] * P
                    nc.sync.dma_start(
                        out=partial[r0:r0 + rows, :]
                            .rearrange("(t p) o -> p t o", p=P),
                        in_=sg[:, :gd_["nt"], :])

            # ---- phase R: ReduceScatter of the partial tables ----
            if "R" in phases:
                nc.gpsimd.collective_compute(
                    "ReduceScatter", mybir.AluOpType.add,
                    replica_groups=[list(range(NCORES))],
                    ins=[partial[:].opt()], outs=[rs_out[:].opt()])

            # ---- phase F: final epilogue on own shard ----
            if "F" in phases:
              with tc.tile_pool(name="fin", bufs=3) as fin:
                FB = 4
                for q in range(math.ceil(n_tiles / FB)):
                    t0_ = q * FB
                    nt = min(FB, n_tiles - t0_)
                    rt = fin.tile([P, FB, out_dim], dt.float16, tag="rt")
                    nc.sync.dma_start(
                        out=rt[:, :nt, :],
                        in_=rs_out[t0_ * P:(t0_ + nt) * P, :]
                            .rearrange("(t p) o -> p t o", p=P))
                    ot = fin.tile([P, FB, out_dim], dt.float32, tag="ot")
                    for j in range(nt):
                        t = t0_ + j
                        nc.vector.tensor_scalar(
                            out=ot[:, j, :], in0=rt[:, j, :],
                            scalar1=dinvc_o_t[:, t:t + 1], scalar2=None,
                            op0=mybir.AluOpType.mult)
                        nc.vector.tensor_tensor(
                            out=ot[:, j, :], in0=ot[:, j, :], in1=b2r_t[:],
                            op=mybir.AluOpType.add)
                    lo = t0_ * P
                    hi = min((t0_ + nt) * P, sh)
                    if hi - lo == nt * P:
                        nc.sync.dma_start(
                            out=t_out[lo:hi, :]
                                .rearrange("(t p) o -> p t o", p=P),
                            in_=ot[:, :nt, :])
                    else:
                        for j in range(nt):
                            tlo = lo + j * P
                            thi = min(tlo + P, sh)
                            if thi <= tlo:
                                break
                            nc.sync.dma_start(
                                out=t_out[tlo:thi, :],
                                in_=ot[:thi - tlo, j, :])

            if debug:
                with tc.tile_pool(name="dbg", bufs=2) as dbp:
                    for t in range(n_tiles):
                        bt = dbp.tile([P, P], dt.float16, tag="b1")
                        nc.sync.dma_start(out=bt[:],
                                          in_=g2_in[t * P:(t + 1) * P, :])
                        nc.sync.dma_start(out=t_dbg_g2[t * P:(t + 1) * P, :],
                                          in_=bt[:])
                        rt = dbp.tile([P, out_dim], dt.float16, tag="b2")
                        nc.sync.dma_start(out=rt[:],
                                          in_=rs_out[t * P:(t + 1) * P, :])
                        nc.sync.dma_start(out=t_dbg_rs[t * P:(t + 1) * P, :],
                                          in_=rt[:])
                    for t in range(n_pad // P):
                        pt = dbp.tile([P, out_dim], dt.float16, tag="b3")
                        nc.sync.dma_start(out=pt[:],
                                          in_=partial[t * P:(t + 1) * P, :])
                        nc.sync.dma_start(out=t_dbg_pt[t * P:(t + 1) * P, :],
                                          in_=pt[:])

    nc.finalize()

    in_maps = []
    for c in range(NCORES):
        in_maps.append({
            "xs": xs, "w1c": w1c, "w2h": w2h, "iota": iota,
            "b1col": b1col, "b2r": b2r,
            "dinvT": dinvT_own[c], "dinv_own": dinv_own[c],
            "dinvc_own": dinvc_own[c],
            "idx1": idx16_1[c], "dloc1": dloc1[c],
            "idx2": idx16_2[c], "dloc2": dloc2[c],
        })

    import time
    t0 = time.monotonic()
    r = run_bass_kernel_spmd(nc, in_maps, list(range(NCORES)))
    _EXEC_STATS["first_call_s"] = time.monotonic() - t0
    _EXEC_STATS["results"] = r
    _EXEC_STATS["nc"] = nc
    _EXEC_STATS["in_maps"] = in_maps

    return np.concatenate([r.results[c]["out"] for c in range(NCORES)],
                          axis=0)


# revision 22
# speedup vs baseline: 3.1005x; 1.2020x over previous
"""MixGCN (2-layer GCN with GReLU mix) on 8 Trainium2 NeuronCores.

Sharding/dataflow (v2):
  L1 (aggregate-first, per-dest-core): z1 = (A~ x) W1. Each core gathers
      dinv-prescaled x rows (fp16, 512B descriptors) for the edges whose
      DESTINATION lies in its node shard, segment-sums them via one-hot
      matmuls into PSUM *transposed* ([k, dest]), then applies W1 with a
      second matmul pair (no transposes anywhere), the GReLU mix epilogue
      on z1^T, and the fused L2 transform g2 = dinv * (h @ W2) for its own
      rows -> local table g2_in.
  L2 (scatter + ReduceScatter): each core aggregates its OWN-SOURCE edges
      toward ALL destinations (one-hot matmuls over gathered g2_in rows),
      writes partial sums [n_pad, 64] fp16, then a ReduceScatter(add)
      hands each core its own destination shard; a tiny epilogue applies
      dinv[dst] and bias.
All loop bounds are static and identical across cores (SPMD); slot counts
per cell are the max over cores, padded slots gather row 0 with an
out-of-range dest id so their one-hot rows are all-zero.
"""
import json
import math
import os

import numpy as np

P = 128
NCORES = 8
import os as _os
GROUP1 = int(_os.environ.get("GCN_G1", "4"))   # dest tiles per L1 group
GROUP2 = int(_os.environ.get("GCN_G2", "8"))   # dest tiles per L2 group
OOB = 30000.0      # dest-local sentinel for padded slots (fp16-exact)
BETA, CMIX = 0.5, 1.0

_EXEC_STATS = {}   # the test harness reads timing info from here


# --------------------------------------------------------------------------
# Workaround for this walrus build's per-instruction sync-wait limit: hoist
# excess immediate semaphore waits onto NoOps inserted before the offending
# instruction on the same engine (sem values are monotonic in-kernel, so
# waiting earlier is equivalent).
# --------------------------------------------------------------------------
_MAXW = 1


def _split_excess_waits(bir: dict) -> dict:
    ctr = 0
    for fn in bir.get("functions", []):
        for bb in fn.get("blocks", []):
            insts = bb.get("instructions", [])
            if not any(
                len(((i.get("sync_info") or {}).get("on_wait") or [])) > _MAXW
                for i in insts
            ):
                continue
            new_insts = []
            for inst in insts:
                si = inst.get("sync_info") or {}
                waits = si.get("on_wait") or []
                if len(waits) > _MAXW:
                    imm = [w for w in waits if w.get("wait_mode") == "sem-ge-imm"]
                    rest = [w for w in waits if w.get("wait_mode") != "sem-ge-imm"]
                    n_keep_imm = max(0, _MAXW - len(rest))
                    hoist = imm[: len(imm) - n_keep_imm]
                    keep = rest + imm[len(imm) - n_keep_imm:]
                    if len(keep) > _MAXW:
                        raise RuntimeError(
                            f"instruction {inst.get('name')} has "
                            f"{len(rest)} non-immediate waits; cannot split")
                    for k in range(0, len(hoist), _MAXW):
                        ctr += 1
                        new_insts.append({
                            "debug": inst.get("debug", 0),
                            "engine": inst["engine"],
                            "ins": [], "outs": [],
                            "name": f"WSPL-{ctr}",
                            "opcode": "NoOp",
                            "sync_info": {"on_update": [],
                                          "on_wait": hoist[k:k + _MAXW]},
                            "text_hint": "wait_split",
                        })
                    si["on_wait"] = keep
                    inst["sync_info"] = si
                new_insts.append(inst)
            bb["instructions"] = new_insts
    return bir


_patched = False


def _install_patch():
    global _patched
    if _patched:
        return
    import concourse.bass as bass
    orig = bass.Bass.to_json_bytes

    def patched(self, *a, **kw):
        bir = json.loads(orig(self, *a, **kw))
        return json.dumps(_split_excess_waits(bir)).encode()

    bass.Bass.to_json_bytes = patched
    _patched = True


# --------------------------------------------------------------------------
# Host-side schedule construction
# --------------------------------------------------------------------------
def _round_up(a, b):
    return ((a + b - 1) // b) * b


def _build_sched(core, row, dst_t, n_tiles, group, blocks):
    """Static SPMD schedule + per-core packed index/dest arrays.

    core[e]:  core that processes edge e
    row[e]:   table row gathered for edge e (global over `blocks`)
    dst_t[e]: destination offset in this layer's padded output space

    Slot layout is IDENTICAL across cores: within each (group, block) cell,
    each destination tile's segment is padded to the max count over cores,
    so one-hot chunk spans are shared and minimal.

    Returns (sched, idx16 [NCORES,128,icols_tot], dloc [NCORES,128,nch_tot]).
    """
    n_groups = math.ceil(n_tiles / group)
    nblk = len(blocks)

    tile_e = dst_t // P
    grp = tile_e // group
    tv_e = tile_e % group
    blk = np.zeros(len(row), dtype=np.int64)
    for bi, (base, size) in enumerate(blocks):
        m = (row >= base) & (row < base + size)
        blk[m] = bi
    order = np.lexsort((dst_t, blk, grp, core))
    row_s, dst_s = row[order], dst_t[order]
    key = (((core[order] * n_groups + grp[order]) * nblk + blk[order])
           * group + tv_e[order])
    nkeys = NCORES * n_groups * nblk * group
    counts_f = np.bincount(key, minlength=nkeys)
    starts_f = np.concatenate([[0], np.cumsum(counts_f)])[:-1]
    counts = counts_f.reshape(NCORES, n_groups, nblk, group)
    starts = starts_f.reshape(NCORES, n_groups, nblk, group)

    seg_len = counts.max(axis=0)                  # [n_groups, nblk, group]
    cell_used = seg_len.sum(axis=2)               # [n_groups, nblk]
    n_slots = np.maximum(_round_up(cell_used, P), P)

    tot_slots = int(n_slots.sum())
    idx16 = np.zeros((NCORES, 16, tot_slots // 16), dtype=np.int16)
    dloc = np.full((NCORES, P, tot_slots // P), OOB, dtype=np.float32)

    sched = []
    icol0 = 0
    chunk0 = 0
    for g in range(n_groups):
        t0 = g * group
        nt = min(group, n_tiles - t0)
        gd = {"t0": t0, "nt": nt, "cells": []}
        for b in range(nblk):
            ns = int(n_slots[g, b])
            nch = ns // P
            base = blocks[b][0]
            segs = seg_len[g, b]                  # [group]
            offs = np.concatenate([[0], np.cumsum(segs)])
            for c in range(NCORES):
                for tv in range(group):
                    cnt = int(counts[c, g, b, tv])
                    if not cnt:
                        continue
                    st = int(starts[c, g, b, tv])
                    loc = (row_s[st:st + cnt] - base).astype(np.int16)
                    dl = (dst_s[st:st + cnt] - t0 * P).astype(np.float32)
                    sl = int(offs[tv]) + np.arange(cnt)
                    idx16[c][sl % 16, icol0 + sl // 16] = loc
                    dloc[c][sl % P, chunk0 + sl // P] = dl
            mm = []
            for k in range(nch):
                lo, hi = k * P, (k + 1) * P
                touched = [tv for tv in range(group)
                           if segs[tv] > 0 and offs[tv] < hi
                           and offs[tv + 1] > lo]
                tv0 = touched[0] if touched else 0
                if tv0:
                    dloc[:, :, chunk0 + k] -= np.float32(tv0 * P)
                mm.append((k, tv0,
                           [[t, False, False] for t in touched]))
            gd["cells"].append({"b": b, "icol0": icol0, "icols": ns // 16,
                                "chunk0": chunk0, "nch": nch, "mm": mm})
            icol0 += ns // 16
            chunk0 += nch
        seen = set()
        last_ref = {}
        for cd in gd["cells"]:
            for _k, _tv0, tl in cd["mm"]:
                for ent in tl:
                    if ent[0] not in seen:
                        ent[1] = True
                        seen.add(ent[0])
                    last_ref[ent[0]] = ent
        for ent in last_ref.values():
            ent[2] = True
        sched.append(gd)

    n_entries = sum(len(tl) for gd in sched for cd in gd["cells"]
                    for _k, _tv0, tl in cd["mm"])
    print(f"  sched: {tot_slots} slots, {tot_slots // P} chunks, "
          f"{n_entries} matmul entries", flush=True)
    return sched, np.tile(idx16, (1, 8, 1)), dloc


def _mk_blocks(total):
    nb = math.ceil(total / 32768)
    bs = _round_up(math.ceil(total / nb), P)
    while bs > 32768:
        nb += 1
        bs = _round_up(math.ceil(total / nb), P)
    blocks = []
    base = 0
    while base < total:
        blocks.append((base, min(bs, total - base)))
        base += bs
    return blocks


# --------------------------------------------------------------------------
# The kernel
# --------------------------------------------------------------------------
def kernel(x, edge_index, W1, b1, ga, gb, gc, gd, W2, b2):
    _install_patch()
    import concourse.bacc as bacc
    import concourse.mybir as mybir
    import concourse.tile as tile
    from concourse.bass_utils import run_bass_kernel_spmd
    from concourse.masks import make_identity

    x = np.asarray(x, dtype=np.float32)
    edge_index = np.asarray(edge_index)
    W1 = np.asarray(W1, dtype=np.float32)
    b1 = np.asarray(b1, dtype=np.float32)
    W2 = np.asarray(W2, dtype=np.float32)
    b2 = np.asarray(b2, dtype=np.float32)
    ga, gb, gc, gd = float(ga), float(gb), float(gc), float(gd)

    n, in_dim = x.shape
    hid = W1.shape[1]
    out_dim = W2.shape[1]
    assert n % NCORES == 0
    sh = n // NCORES
    n_tiles = math.ceil(sh / P)          # own-shard dest tiles (98)
    sh_pad = n_tiles * P                 # 12544
    n_pad = sh_pad * NCORES              # padded global rows (100352)
    n_tiles_g = n_pad // P               # global dest tiles (784)
    kchunks = in_dim // P                # 2

    src = np.concatenate([edge_index[0].astype(np.int64),
                          np.arange(n, dtype=np.int64)])
    dst = np.concatenate([edge_index[1].astype(np.int64),
                          np.arange(n, dtype=np.int64)])

    deg = np.bincount(dst, minlength=n).astype(np.float32)
    dinv = np.where(deg > 0, 1.0 / np.sqrt(deg), 0.0).astype(np.float32)

    # ---- L1 schedule: by destination core; table = xs (padded to n_rows1)
    n_rows1 = _round_up(n, P)
    blocks1 = _mk_blocks(n_rows1)
    sched1, idx16_1, dloc1 = _build_sched(
        dst // sh, src, dst % sh, n_tiles, GROUP1, blocks1)

    # ---- L2 schedule: by source core; table = own g2_in (sh_pad rows);
    #      dest space = padded global rows
    dst_pad = (dst // sh) * sh_pad + (dst % sh)
    sched2, idx16_2, dloc2 = _build_sched(
        src // sh, src % sh, dst_pad, n_tiles_g, GROUP2, [(0, sh_pad)])

    # GReLU mix: h = z * (k0 + k1*[z<0] + k2*[z>=kc])
    k0 = BETA + (CMIX - BETA) * gb
    k1 = (CMIX - BETA) * (ga - gb)
    k2 = (CMIX - BETA) * (gd - gb)
    kc = gc

    # ---- host-prepared dense arrays
    xs = np.zeros((n_rows1, in_dim), dtype=np.float16)
    xs[:n] = (dinv[:, None] * x).astype(np.float16)
    w1c = np.ascontiguousarray(
        W1.reshape(kchunks, P, hid).transpose(1, 0, 2)).astype(np.float16)
    w2h = W2.astype(np.float16)                          # [hid, out]
    iota = np.tile(np.arange(GROUP2 * P, dtype=np.float16), (P, 1))
    b1col = b1.reshape(hid, 1).astype(np.float32)
    b2r = np.tile(CMIX * b2, (P, 1)).astype(np.float32)

    dinvT_own = np.zeros((NCORES, P, sh_pad), dtype=np.float16)
    dinv_own = np.zeros((NCORES, P, n_tiles), dtype=np.float32)
    for c in range(NCORES):
        dv = np.zeros(sh_pad, dtype=np.float32)
        dv[:sh] = dinv[c * sh:(c + 1) * sh]
        dinvT_own[c] = np.tile(dv.astype(np.float16), (P, 1))
        dinv_own[c] = dv.reshape(n_tiles, P).T
    dinvc_own = (CMIX * dinv_own).astype(np.float32)

    max_icols1 = max(cd["icols"] for g in sched1 for cd in g["cells"])
    max_nch1 = max(cd["nch"] for g in sched1 for cd in g["cells"])
    max_icols2 = max(cd["icols"] for g in sched2 for cd in g["cells"])
    max_nch2 = max(cd["nch"] for g in sched2 for cd in g["cells"])

    # ---- build the bass program ----
    nc = bacc.Bacc()
    dt = mybir.dt
    t_xs = nc.dram_tensor("xs", [n_rows1, in_dim], dt.float16,
                          kind="ExternalInput")
    t_w1c = nc.dram_tensor("w1c", [P, kchunks, hid], dt.float16,
                           kind="ExternalInput")
    t_w2h = nc.dram_tensor("w2h", [hid, out_dim], dt.float16,
                           kind="ExternalInput")
    t_iota = nc.dram_tensor("iota", [P, GROUP2 * P], dt.float16,
                            kind="ExternalInput")
    t_b1col = nc.dram_tensor("b1col", [hid, 1], dt.float32,
                             kind="ExternalInput")
    t_b2r = nc.dram_tensor("b2r", [P, out_dim], dt.float32,
                           kind="ExternalInput")
    t_dinvT = nc.dram_tensor("dinvT", [P, sh_pad], dt.float16,
                             kind="ExternalInput")
    t_dinv_own = nc.dram_tensor("dinv_own", [P, n_tiles], dt.float32,
                                kind="ExternalInput")
    t_dinvc_own = nc.dram_tensor("dinvc_own", [P, n_tiles], dt.float32,
                                 kind="ExternalInput")
    t_idx1 = nc.dram_tensor("idx1", list(idx16_1.shape[1:]), dt.int16,
                            kind="ExternalInput")
    t_dloc1 = nc.dram_tensor("dloc1", list(dloc1.shape[1:]), dt.float32,
                             kind="ExternalInput")
    t_idx2 = nc.dram_tensor("idx2", list(idx16_2.shape[1:]), dt.int16,
                            kind="ExternalInput")
    t_dloc2 = nc.dram_tensor("dloc2", list(dloc2.shape[1:]), dt.float32,
                             kind="ExternalInput")
    t_out = nc.dram_tensor("out", [sh, out_dim], dt.float32,
                           kind="ExternalOutput")
    debug = bool(os.environ.get("GCN_DEBUG"))
    if debug:
        t_dbg_g2 = nc.dram_tensor("dbg_g2", [sh_pad, P], dt.float16,
                                  kind="ExternalOutput")
        t_dbg_pt = nc.dram_tensor("dbg_pt", [n_pad, out_dim], dt.float16,
                                  kind="ExternalOutput")
        t_dbg_rs = nc.dram_tensor("dbg_rs", [sh_pad, out_dim], dt.float16,
                                  kind="ExternalOutput")
        t_dbg_ax = nc.dram_tensor("dbg_ax", [sh_pad, in_dim], dt.float16,
                                  kind="ExternalOutput")
        t_dbg_z = nc.dram_tensor("dbg_z", [sh_pad, hid], dt.float16,
                                 kind="ExternalOutput")

    phases = os.environ.get("GCN_PHASES", "BDRF")

    with tile.TileContext(nc, pool_alloc_mode="queue") as tc:
        with (
            tc.tile_pool(name="dram", bufs=1, space="DRAM") as dram,
            tc.tile_pool(name="consts", bufs=1) as cpool,
        ):
            g2_in = dram.tile([sh_pad, P], dt.float16)   # cols 64: junk
            partial = dram.tile([n_pad, out_dim], dt.float16)
            rs_out = dram.tile([sh_pad, out_dim], dt.float16)

            w1_t = cpool.tile([P, kchunks, hid], dt.float16)
            nc.sync.dma_start(out=w1_t[:], in_=t_w1c[:])
            w2_t = cpool.tile([hid, out_dim], dt.float16)
            nc.sync.dma_start(out=w2_t[:], in_=t_w2h[:])
            iota_t = cpool.tile([P, GROUP2 * P], dt.float16)
            nc.sync.dma_start(out=iota_t[:], in_=t_iota[:])
            b1c_t = cpool.tile([hid, 1], dt.float32)
            nc.sync.dma_start(out=b1c_t[:], in_=t_b1col[:])
            b2r_t = cpool.tile([P, out_dim], dt.float32)
            nc.sync.dma_start(out=b2r_t[:], in_=t_b2r[:])
            dinvT_t = cpool.tile([P, sh_pad], dt.float16)
            nc.sync.dma_start(out=dinvT_t[:], in_=t_dinvT[:])
            dinv_o_t = cpool.tile([P, n_tiles], dt.float32)
            nc.sync.dma_start(out=dinv_o_t[:], in_=t_dinv_own[:])
            dinvc_o_t = cpool.tile([P, n_tiles], dt.float32)
            nc.sync.dma_start(out=dinvc_o_t[:], in_=t_dinvc_own[:])
            dloc1_t = cpool.tile([P, dloc1.shape[2]], dt.float32)
            nc.sync.dma_start(out=dloc1_t[:], in_=t_dloc1[:])
            dloc2_t = cpool.tile([P, dloc2.shape[2]], dt.float32)
            nc.sync.dma_start(out=dloc2_t[:], in_=t_dloc2[:])
            ident_t = cpool.tile([P, P], dt.float16)
            make_identity(nc, ident_t[:])

            # ---- phase B: L1 gather + transposed segment-sum + dense +
            #      GReLU mix + fused L2 transform ----
            if "B" in phases:
              with (
                tc.tile_pool(name="gat1", bufs=3) as gat,
                tc.tile_pool(name="gidx1", bufs=3) as gidx,
                tc.tile_pool(name="smat1", bufs=8) as smat,
                tc.tile_pool(name="seg_ps1", bufs=GROUP1 + 1,
                             space="PSUM") as sps,
                tc.tile_pool(name="z_ps", bufs=2, space="PSUM") as zps,
                tc.tile_pool(name="t_ps", bufs=1, space="PSUM") as tps,
                tc.tile_pool(name="epi1", bufs=10) as epool,
              ):
                for gd_ in sched1:
                    psums = {}
                    for cd in gd_["cells"]:
                        ns = cd["nch"] * P
                        nch = cd["nch"]
                        it = gidx.tile([P, max_icols1], dt.int16, tag="idx")
                        nc.sync.dma_start(
                            out=it[:, :cd["icols"]],
                            in_=t_idx1[:, cd["icol0"]:cd["icol0"] + cd["icols"]])
                        gt = gat.tile([P, max_nch1, in_dim], dt.float16,
                                      tag="g")
                        base, bsize = blocks1[cd["b"]]
                        nc.gpsimd.dma_gather(
                            out_ap=gt[:, :nch, :],
                            in_ap=t_xs[base:base + bsize, :],
                            idxs_ap=it[:, :cd["icols"]],
                            num_idxs=ns, num_idxs_reg=ns, elem_size=in_dim,
                            single_packet=False)
                        for kloc, tv0, tlist in cd["mm"]:
                            ch = cd["chunk0"] + kloc
                            for (tv, first, last) in tlist:
                                if tv not in psums:
                                    psums[tv] = sps.tile(
                                        [P, in_dim], dt.float32,
                                        space="PSUM", tag="segps1",
                                        name=f"ps1_{tv}")
                                rel = tv - tv0
                                s_x = smat.tile([P, P], dt.float16, tag="sx")
                                nc.vector.tensor_scalar(
                                    out=s_x[:],
                                    in0=iota_t[:, rel * P:(rel + 1) * P],
                                    scalar1=dloc1_t[:, ch:ch + 1],
                                    scalar2=None,
                                    op0=mybir.AluOpType.is_equal)
                                nc.tensor.matmul(
                                    out=psums[tv][:],
                                    lhsT=s_x[:],
                                    rhs=gt[:, kloc, :],
                                    start=first, stop=last)
                    touched = sorted(
                        tv for _cd in gd_["cells"]
                        for _k, _tv0, _tl in _cd["mm"] for (tv, _f, _l) in _tl)
                    for tv in sorted(set(touched)):
                        t = gd_["t0"] + tv
                        if t >= n_tiles:
                            continue
                        ax = epool.tile([P, in_dim], dt.float16, tag="ax")
                        nc.scalar.copy(out=ax[:], in_=psums[tv][:])
                        axT = epool.tile([P, kchunks, P], dt.float16,
                                         tag="axT")
                        for kk in range(kchunks):
                            tp = tps.tile([P, P], dt.float16, space="PSUM",
                                          tag="tps")
                            nc.tensor.transpose(
                                out=tp[:], in_=ax[:, kk * P:(kk + 1) * P],
                                identity=ident_t[:])
                            nc.scalar.copy(out=axT[:, kk, :], in_=tp[:])
                        zb = zps.tile([P, P + out_dim], dt.float32,
                                      space="PSUM", tag="zps")
                        zp = zb[:, :P]
                        for kk in range(kchunks):
                            nc.tensor.matmul(
                                out=zp, lhsT=w1_t[:, kk, :],
                                rhs=axT[:, kk, :],
                                start=(kk == 0), stop=(kk == kchunks - 1))
                        # epilogue on z1^T [h, dest]
                        zt = epool.tile([P, P], dt.float16, tag="zt")
                        nc.vector.tensor_tensor(
                            out=zt[:], in0=zp,
                            in1=dinvT_t[:, t * P:(t + 1) * P],
                            op=mybir.AluOpType.mult)
                        nc.vector.tensor_scalar(
                            out=zt[:], in0=zt[:], scalar1=b1c_t[:, 0:1],
                            scalar2=None, op0=mybir.AluOpType.add)
                        if debug:
                            nc.sync.dma_start(
                                out=t_dbg_ax[t * P:(t + 1) * P, :],
                                in_=ax[:])
                            nc.sync.dma_start(
                                out=t_dbg_z[t * P:(t + 1) * P, :],
                                in_=zt[:])
                        mn = epool.tile([P, P], dt.float16, tag="mn")
                        nc.vector.tensor_scalar(
                            out=mn[:], in0=zt[:], scalar1=0.0, scalar2=None,
                            op0=mybir.AluOpType.is_lt)
                        mh = epool.tile([P, P], dt.float16, tag="mh")
                        nc.vector.tensor_scalar(
                            out=mh[:], in0=zt[:], scalar1=kc, scalar2=None,
                            op0=mybir.AluOpType.is_ge)
                        nc.vector.tensor_scalar(
                            out=mn[:], in0=mn[:], scalar1=k1, scalar2=k0,
                            op0=mybir.AluOpType.mult,
                            op1=mybir.AluOpType.add)
                        nc.vector.tensor_scalar(
                            out=mh[:], in0=mh[:], scalar1=k2, scalar2=None,
                            op0=mybir.AluOpType.mult)
                        nc.vector.tensor_tensor(
                            out=mn[:], in0=mn[:], in1=mh[:],
                            op=mybir.AluOpType.add)
                        ht = epool.tile([P, P], dt.float16, tag="ht")
                        nc.vector.tensor_tensor(
                            out=ht[:], in0=zt[:], in1=mn[:],
                            op=mybir.AluOpType.mult)
                        # fused L2 transform: g2 = dinv * (h @ W2)
                        gp = zb[:, P:P + out_dim]
                        nc.tensor.matmul(out=gp, lhsT=ht[:], rhs=w2_t[:],
                                         start=True, stop=True)
                        g2t = epool.tile([P, out_dim], dt.float16, tag="g2t")
                        nc.vector.tensor_scalar(
                            out=g2t[:], in0=gp,
                            scalar1=dinv_o_t[:, t:t + 1], scalar2=None,
                            op0=mybir.AluOpType.mult)
                        nc.scalar.dma_start(
                            out=g2_in[t * P:(t + 1) * P, :out_dim],
                            in_=g2t[:])

            # ---- phase D: L2 scatter partial sums over ALL destinations ----
            if "D" in phases:
              with (
                tc.tile_pool(name="gat2", bufs=3) as gat,
                tc.tile_pool(name="gidx2", bufs=3) as gidx,
                tc.tile_pool(name="smat2", bufs=8) as smat,
                tc.tile_pool(name="seg_ps2", bufs=GROUP2,
                             space="PSUM") as sps,
                tc.tile_pool(name="stage2", bufs=3) as stg,
              ):
                for gi, gd_ in enumerate(sched2):
                    sg = stg.tile([P, GROUP2, out_dim], dt.float16, tag="sg")
                    psums = {}
                    for cd in gd_["cells"]:
                        ns = cd["nch"] * P
                        nch = cd["nch"]
                        it = gidx.tile([P, max_icols2], dt.int16, tag="idx")
                        nc.sync.dma_start(
                            out=it[:, :cd["icols"]],
                            in_=t_idx2[:, cd["icol0"]:cd["icol0"] + cd["icols"]])
                        gt = gat.tile([P, max_nch2, P], dt.float16, tag="g")
                        nc.gpsimd.dma_gather(
                            out_ap=gt[:, :nch, :],
                            in_ap=g2_in[:, :],
                            idxs_ap=it[:, :cd["icols"]],
                            num_idxs=ns, num_idxs_reg=ns, elem_size=P,
                            single_packet=False)
                        for kloc, tv0, tlist in cd["mm"]:
                            ch = cd["chunk0"] + kloc
                            for (tv, first, last) in tlist:
                                if tv not in psums:
                                    psums[tv] = sps.tile(
                                        [P, out_dim], dt.float32,
                                        space="PSUM", tag="segps2",
                                        name=f"ps2_{tv}")
                                rel = tv - tv0
                                s_x = smat.tile([P, P], dt.float16, tag="sx")
                                nc.vector.tensor_scalar(
                                    out=s_x[:],
                                    in0=iota_t[:, rel * P:(rel + 1) * P],
                                    scalar1=dloc2_t[:, ch:ch + 1],
                                    scalar2=None,
                                    op0=mybir.AluOpType.is_equal)
                                nc.tensor.matmul(
                                    out=psums[tv][:],
                                    lhsT=s_x[:],
                                    rhs=gt[:, kloc, :out_dim],
                                    start=first, stop=last)
                    touched = set(
                        tv for _cd in gd_["cells"]
                        for _k, _tv0, _tl in _cd["mm"] for (tv, _f, _l) in _tl)
                    for tv in range(gd_["nt"]):
                        if tv in touched:
                            nc.scalar.copy(out=sg[:, tv, :],
                                           in_=psums[tv][:])
                        else:
                            nc.scalar.memzero(sg[:, tv, :])
                    r0 = gd_["t0"] * P
                    rows = gd_["nt"] * P
                    nc.scalar.dma_start(
                        out=partial[r0:r0 + rows, :]
                            .rearrange("(t p) o -> p t o", p=P),
                        in_=sg[:, :gd_["nt"], :])

            # ---- phase R: ReduceScatter of the partial tables ----
            if "R" in phases:
                nc.gpsimd.collective_compute(
                    "ReduceScatter", mybir.AluOpType.add,
                    replica_groups=[list(range(NCORES))],
                    ins=[partial[:].opt()], outs=[rs_out[:].opt()])

            # ---- phase F: final epilogue on own shard ----
            if "F" in phases:
              with tc.tile_pool(name="fin", bufs=3) as fin:
                FB = 4
                for q in range(math.ceil(n_tiles / FB)):
                    t0_ = q * FB
                    nt = min(FB, n_tiles - t0_)
                    rt = fin.tile([P, FB, out_dim], dt.float16, tag="rt")
                    nc.sync.dma_start(
                        out=rt[:, :nt, :],
                        in_=rs_out[t0_ * P:(t0_ + nt) * P, :]
                            .rearrange("(t p) o -> p t o", p=P))
                    ot = fin.tile([P, FB, out_dim], dt.float32, tag="ot")
                    for j in range(nt):
                        t = t0_ + j
                        nc.vector.tensor_scalar(
                            out=ot[:, j, :], in0=rt[:, j, :],
                            scalar1=dinvc_o_t[:, t:t + 1], scalar2=None,
                            op0=mybir.AluOpType.mult)
                        nc.vector.tensor_tensor(
                            out=ot[:, j, :], in0=ot[:, j, :], in1=b2r_t[:],
                            op=mybir.AluOpType.add)
                    lo = t0_ * P
                    hi = min((t0_ + nt) * P, sh)
                    if hi - lo == nt * P:
                        nc.scalar.dma_start(
                            out=t_out[lo:hi, :]
                                .rearrange("(t p) o -> p t o", p=P),
                            in_=ot[:, :nt, :])
                    else:
                        for j in range(nt):
                            tlo = lo + j * P
                            thi = min(tlo + P, sh)
                            if thi <= tlo:
                                break
                            nc.scalar.dma_start(
                                out=t_out[tlo:thi, :],
                                in_=ot[:thi - tlo, j, :])

            if debug:
                with tc.tile_pool(name="dbg", bufs=2) as dbp:
                    for t in range(n_tiles):
                        bt = dbp.tile([P, P], dt.float16, tag="b1")
                        nc.sync.dma_start(out=bt[:],
                                          in_=g2_in[t * P:(t + 1) * P, :])
                        nc.sync.dma_start(out=t_dbg_g2[t * P:(t + 1) * P, :],
                                          in_=bt[:])
                        rt = dbp.tile([P, out_dim], dt.float16, tag="b2")
                        nc.sync.dma_start(out=rt[:],
                                          in_=rs_out[t * P:(t + 1) * P, :])
                        nc.sync.dma_start(out=t_dbg_rs[t * P:(t + 1) * P, :],
                                          in_=rt[:])
                    for t in range(n_pad // P):
                        pt = dbp.tile([P, out_dim], dt.float16, tag="b3")
                        nc.sync.dma_start(out=pt[:],
                                          in_=partial[t * P:(t + 1) * P, :])
                        nc.sync.dma_start(out=t_dbg_pt[t * P:(t + 1) * P, :],
                                          in_=pt[:])

    nc.finalize()

    in_maps = []
    for c in range(NCORES):
        in_maps.append({
            "xs": xs, "w1c": w1c, "w2h": w2h, "iota": iota,
            "b1col": b1col, "b2r": b2r,
            "dinvT": dinvT_own[c], "dinv_own": dinv_own[c],
            "dinvc_own": dinvc_own[c],
            "idx1": idx16_1[c], "dloc1": dloc1[c],
            "idx2": idx16_2[c], "dloc2": dloc2[c],
        })

    import time
    t0 = time.monotonic()
    r = run_bass_kernel_spmd(nc, in_maps, list(range(NCORES)))
    _EXEC_STATS["first_call_s"] = time.monotonic() - t0
    _EXEC_STATS["results"] = r
    _EXEC_STATS["nc"] = nc
    _EXEC_STATS["in_maps"] = in_maps

    return np.concatenate([r.results[c]["out"] for c in range(NCORES)],
                          axis=0)
